# revision 13
# baseline (speedup 1.0000x reference)
# Trainium2 Bass kernel for nn_Decoder (attention + GRUCell decode loop).
#
# Sharding: pure data parallel over the batch dim across 8 NeuronCores.
# Each core processes B/8 = 8192 batch elements; weights are replicated.
#
# Per-core layout strategy:
#   - batch processed in chunks of 512 (4 "waves" of 128 partitions)
#   - attention (scores/softmax/context) computed batch-major on DVE/ACT
#   - GRU matmuls on the PE in feature-major layout (weights stationary),
#     with PE transposes moving hidden between layouts
#   - the `last` feedback window is kept as a 3-row ring buffer so no
#     cross-partition shifts are ever needed (rotated W_ih stationaries)

import os
import numpy as np

import concourse.bacc as bacc
import concourse.bass as bass
import concourse.mybir as mybir
import concourse.tile as tile
from concourse.masks import make_identity

F32 = mybir.dt.float32
AX = mybir.AxisListType
OP = mybir.AluOpType
AF = mybir.ActivationFunctionType

T_IN = 12
T_OUT = 12
H = 64
OV = 3
B = 65536
NCORES = 8
P = 128  # sbuf partitions per wave
NW = 4  # waves per chunk
CHUNK = P * NW  # 512


def _dap(x: bass.AP, dims, off=0):
    """Custom access pattern over x's tensor: explicit [step, count] dims."""
    return bass.AP(tensor=x.tensor, offset=x.offset + off, ap=[list(d) for d in dims])


def _bap(t, dims):
    """AP over tile t keeping its partition dim, custom free dims."""
    a = t[:] if not isinstance(t, bass.AP) else t
    return bass.AP(
        tensor=a.tensor, offset=a.offset, ap=[list(a.ap[0])] + [list(d) for d in dims]
    )


def decoder_body(ctx, tc, enc, hid, last, w_ih, w_hh, b_ih, b_hh, w_lin, b_lin, out, bc):
    nc = tc.nc
    n_chunks = bc // CHUNK

    consts = ctx.enter_context(tc.tile_pool(name="consts", bufs=1))
    encp = ctx.enter_context(tc.tile_pool(name="encp", bufs=2))
    tmpp = ctx.enter_context(tc.tile_pool(name="tmpp", bufs=2))
    statep = ctx.enter_context(tc.tile_pool(name="statep", bufs=3))
    workp = ctx.enter_context(tc.tile_pool(name="workp", bufs=3))
    outp = ctx.enter_context(tc.tile_pool(name="outp", bufs=2))
    psump = ctx.enter_context(tc.tile_pool(name="psump", bufs=1, space="PSUM"))

    # ---------------- constants ----------------
    ident = consts.tile([P, P], F32, tag="ident")
    make_identity(nc, ident)

    # W_hh (192, 64) -> whhT (64, 192) so whhT[:, g*64:(g+1)*64] is the lhsT
    # (K=64, M=64) for gate g's hidden matmul.
    whhT = consts.tile([H, 3 * H], F32, tag="whhT")
    nc.sync.dma_start(out=whhT, in_=_dap(w_hh, [[1, H], [H, 3 * H]]))

    # W_ih (192, 3) -> three row-rotated copies of its transpose (3, 192).
    # At step s (m = s % 3) ring row r must be multiplied by W_ih column
    # (m - 1 - r) % 3.
    wihrot = consts.tile([OV, 3, 3 * H], F32, tag="wihrot")
    for m in range(3):
        for r in range(OV):
            i = (m - 1 - r) % 3
            nc.sync.dma_start(
                out=wihrot[r : r + 1, m, :],
                in_=_dap(w_ih, [[1, 1], [OV, 3 * H]], off=i),
            )

    # biases as (64, 3) feature-major tiles: column g = gate g
    bihs = consts.tile([H, 3], F32, tag="bihs")
    nc.sync.dma_start(out=bihs, in_=_dap(b_ih, [[1, H], [H, 3]]))
    bhhs = consts.tile([H, 3], F32, tag="bhhs")
    nc.sync.dma_start(out=bhhs, in_=_dap(b_hh, [[1, H], [H, 3]]))


    # W_lin (1, 64) -> (64, 1), plus 3 rotated (64, 3) variants with the
    # column placed at ring slot m (others zero) for the ring update matmul.
    wlinT = consts.tile([H, 1], F32, tag="wlinT")
    nc.sync.dma_start(out=wlinT, in_=_dap(w_lin, [[1, H], [H, 1]]))
    wlin3 = consts.tile([H, 3, 3], F32, tag="wlin3")
    nc.vector.memset(wlin3, 0.0)
    for m in range(3):
        nc.vector.tensor_copy(out=wlin3[:, m, m : m + 1], in_=wlinT)

    blin = consts.tile([1, 1], F32, tag="blin")
    nc.sync.dma_start(out=blin, in_=_dap(b_lin, [[1, 1], [1, 1]]))

    # The ring stores y WITHOUT b_lin (so the ring update is a single fused
    # op).  The missing b_lin is folded into the gate biases: at step s the
    # ring holds min(s, 3) y-type entries, each contributing
    # b_lin * W_ih[:, age] to the pre-activations.
    # bias_var[k, v, g]: v = min(s, 3); g in {r, z, n}; for r/z it includes
    # b_ih+b_hh, for n only b_ih (b_hh_n is applied in the hn Identity op).
    blin64 = consts.tile([H, 1], F32, tag="blin64")
    nc.sync.dma_start(out=blin64, in_=_dap(b_lin, [[0, H], [1, 1]]))
    wcol = consts.tile([H, OV, 3], F32, tag="wcol")  # [k, age i, gate]
    for i in range(OV):
        nc.sync.dma_start(
            out=wcol[:, i, :], in_=_dap(w_ih, [[OV, H], [OV * H, 3]], off=i)
        )
    bias_var = consts.tile([H, 4, 3], F32, tag="bias_var")
    nc.vector.tensor_copy(out=bias_var[:, 0, :], in_=bihs)
    nc.vector.tensor_add(
        out=bias_var[:, 0, 0:2], in0=bias_var[:, 0, 0:2], in1=bhhs[:, 0:2]
    )
    for v in range(1, 4):
        nc.vector.scalar_tensor_tensor(
            out=bias_var[:, v, :],
            in0=wcol[:, v - 1, :],
            scalar=blin64,
            in1=bias_var[:, v - 1, :],
            op0=OP.mult,
            op1=OP.add,
        )

    # mask3[:, m]: 0 at slot m, 1 elsewhere (ring update kills old slot m)
    mask3 = consts.tile([OV, 3], F32, tag="mask3")
    nc.gpsimd.memset(mask3, 1.0)
    # mask3[x, y] = (x - y) != 0 ? 1.0 : 0.0
    nc.gpsimd.affine_select(
        out=mask3,
        in_=mask3,
        compare_op=OP.not_equal,
        fill=0.0,
        base=0,
        pattern=[[-1, 3]],
        channel_multiplier=1,
    )

    # ---------------- main loop ----------------
    for c in range(n_chunks):
        b0 = c * CHUNK

        # enc chunk: (128 p, 4 w, 12 t, 64 k); batch index = b0 + w*128 + p
        enc_b = encp.tile([P, NW, T_IN, H], F32, tag="enc")
        for w in range(NW):
            nc.sync.dma_start(
                out=enc_b[:, w, :, :],
                in_=_dap(
                    enc, [[H, P], [bc * H, T_IN], [1, H]], off=(b0 + w * P) * H
                ),
            )

        # initial hidden, batch-major (128, 4, 64)
        h_b = statep.tile([P, NW, H], F32, tag="h_b")
        nc.sync.dma_start(
            out=h_b, in_=_dap(hid, [[H, P], [P * H, NW], [1, H]], off=b0 * H)
        )
        # initial hidden, feature-major (64, 512)
        ps_t0 = psump.tile([H, CHUNK], F32, tag="ps_hT")
        for w in range(NW):
            nc.tensor.transpose(ps_t0[:, w * P : (w + 1) * P], h_b[:, w, :], ident)
        hT = statep.tile([H, CHUNK], F32, tag="hT")
        nc.scalar.copy(out=hT, in_=ps_t0)

        # last-window ring (3, 512): ring[(-1 - i) % 3] = last[:, i]
        lst = statep.tile([OV, CHUNK], F32, tag="lst")
        for i in range(OV):
            slot = (-1 - i) % 3
            nc.sync.dma_start(
                out=lst[slot : slot + 1, :],
                in_=_dap(last, [[1, 1], [OV, CHUNK]], off=b0 * OV + i),
            )

        y_all = outp.tile([1, T_OUT, CHUNK], F32, tag="y_all")

        for s in range(T_OUT):
            m = s % 3
            bv = min(s, 3)

            # ---- attention scores: s[p,w,t] = sum_k enc*h ----
            # (multiplies on GpSimd, reductions on DVE — DVE is the
            # bottleneck engine, POOL is otherwise idle)
            tmp = tmpp.tile([P, NW, T_IN, H], F32, tag="tmp")
            nc.vector.tensor_mul(
                out=tmp[:, 0:2],
                in0=enc_b[:, 0:2],
                in1=_bap(h_b[:, 0:2, :], [[H, 2], [0, T_IN], [1, H]]),
            )
            _meng = nc.gpsimd if os.environ.get("ATTN_POOL", "1") == "1" else nc.vector
            _meng.tensor_mul(
                out=tmp[:, 2:4],
                in0=enc_b[:, 2:4],
                in1=_bap(h_b[:, 2:4, :], [[H, 2], [0, T_IN], [1, H]]),
            )
            sc = workp.tile([P, NW, T_IN], F32, tag="sc")
            nc.vector.tensor_reduce(out=sc, in_=tmp, axis=AX.X, op=OP.add)

            # ---- softmax over t ----
            nmax = workp.tile([P, NW], F32, tag="nmax")
            nc.vector.tensor_reduce(
                out=nmax, in_=sc, axis=AX.X, op=OP.max, negate=True
            )
            e = workp.tile([P, NW, T_IN], F32, tag="e")
            nc.vector.tensor_add(
                out=e, in0=sc, in1=_bap(nmax, [[1, NW], [0, T_IN]])
            )
            nc.scalar.activation(out=e, in_=e, func=AF.Exp)
            ssum = workp.tile([P, NW], F32, tag="ssum")
            nc.vector.tensor_reduce(out=ssum, in_=e, axis=AX.X, op=OP.add)
            nc.vector.reciprocal(out=ssum, in_=ssum)
            wgt = workp.tile([P, NW, T_IN], F32, tag="wgt")
            nc.vector.tensor_mul(
                out=wgt, in0=e, in1=_bap(ssum, [[1, NW], [0, T_IN]])
            )

            # ---- context: ctx[p,w,k] = sum_t enc * wgt ----
            # ---- context, fused with the transpose to feature-major ----
            # q[p,w,t,k] = wgt * enc (elementwise); the sum over t is done by
            # the PE: 12 transpose-matmuls per wave accumulate q_t^T into the
            # same PSUM columns, yielding ctx^T (64, 512) with no DVE reduce.
            tmp2 = tmpp.tile([P, NW, T_IN, H], F32, tag="tmp")
            _meng.tensor_mul(
                out=tmp2,
                in0=enc_b,
                in1=_bap(wgt, [[T_IN, NW], [1, T_IN], [0, H]]),
            )
            ps_ct = psump.tile([H, CHUNK], F32, tag="ps_hT")
            for w in range(NW):
                for t in range(T_IN):
                    nc.tensor.matmul(
                        ps_ct[:, w * P : (w + 1) * P],
                        tmp2[:, w, t, :],
                        ident,
                        start=(t == 0),
                        stop=(t == T_IN - 1),
                        is_transpose=True,
                    )
            hattT = statep.tile([H, CHUNK], F32, tag="hattT")
            nc.vector.tensor_add(out=hattT, in0=hT, in1=ps_ct)

            # ---- GRU gates ----
            ps_r = psump.tile([H, CHUNK], F32, tag="ps_r")
            nc.tensor.matmul(ps_r, whhT[:, 0:H], hattT, start=True, stop=False)
            nc.tensor.matmul(ps_r, wihrot[:, m, 0:H], lst, start=False, stop=True)
            ps_z = psump.tile([H, CHUNK], F32, tag="ps_z")
            nc.tensor.matmul(ps_z, whhT[:, H : 2 * H], hattT, start=True, stop=False)
            nc.tensor.matmul(
                ps_z, wihrot[:, m, H : 2 * H], lst, start=False, stop=True
            )
            r_s = workp.tile([H, CHUNK], F32, tag="r_s")
            nc.scalar.activation(
                out=r_s, in_=ps_r, func=AF.Sigmoid, bias=bias_var[:, bv, 0:1], scale=1.0
            )
            z_s = workp.tile([H, CHUNK], F32, tag="z_s")
            nc.scalar.activation(
                out=z_s, in_=ps_z, func=AF.Sigmoid, bias=bias_var[:, bv, 1:2], scale=1.0
            )

            ps_n2 = psump.tile([H, CHUNK], F32, tag="ps_n2")
            nc.tensor.matmul(
                ps_n2, whhT[:, 2 * H : 3 * H], hattT, start=True, stop=True
            )
            ps_n1 = psump.tile([H, CHUNK], F32, tag="ps_n1")
            nc.tensor.matmul(
                ps_n1, wihrot[:, m, 2 * H : 3 * H], lst, start=True, stop=True
            )
            hn = workp.tile([H, CHUNK], F32, tag="hn")
            nc.scalar.activation(
                out=hn, in_=ps_n2, func=AF.Identity, bias=bhhs[:, 2:3], scale=1.0
            )
            u = workp.tile([H, CHUNK], F32, tag="u")
            nc.vector.tensor_mul(out=u, in0=r_s, in1=hn)
            nc.vector.tensor_add(out=u, in0=u, in1=ps_n1)
            n_t = workp.tile([H, CHUNK], F32, tag="n_t")
            nc.scalar.activation(
                out=n_t, in_=u, func=AF.Tanh, bias=bias_var[:, bv, 2:3], scale=1.0
            )
            # h' = n + z * (h_att - n)
            v = workp.tile([H, CHUNK], F32, tag="v")
            nc.vector.tensor_sub(out=v, in0=hattT, in1=n_t)
            nc.vector.tensor_mul(out=v, in0=z_s, in1=v)
            hT_new = statep.tile([H, CHUNK], F32, tag="hT")
            nc.vector.tensor_add(out=hT_new, in0=n_t, in1=v)

            # ---- output y = h' @ W_lin.T + b_lin ----
            ps_y = psump.tile([1, CHUNK], F32, tag="ps_y")
            nc.tensor.matmul(ps_y, wlinT, hT_new, start=True, stop=True)
            nc.scalar.activation(
                out=y_all[:, s, :], in_=ps_y, func=AF.Identity, bias=blin, scale=1.0
            )

            # ---- ring update: slot m <- y ----
            ps_g = psump.tile([OV, CHUNK], F32, tag="ps_g")
            nc.tensor.matmul(ps_g, wlin3[:, m, :], hT_new, start=True, stop=True)
            lst_new = statep.tile([OV, CHUNK], F32, tag="lst")
            nc.vector.scalar_tensor_tensor(
                out=lst_new,
                in0=lst,
                scalar=mask3[:, m : m + 1],
                in1=ps_g,
                op0=OP.mult,
                op1=OP.add,
            )

            # ---- h' back to batch-major for next step's scores ----
            if s < T_OUT - 1:
                ps_hb = psump.tile([P, NW * H], F32, tag="ps_hb")
                for w in range(NW):
                    nc.tensor.transpose(
                        ps_hb[:, w * H : (w + 1) * H],
                        hT_new[:, w * P : (w + 1) * P],
                        ident[0:H, 0:H],
                    )
                h_b = statep.tile([P, NW, H], F32, tag="h_b")
                nc.scalar.copy(out=h_b, in_=ps_hb)

            hT = hT_new
            lst = lst_new

        # ---- write chunk outputs: y_all (1, 12, 512) -> out[b0:b0+512, :] ----
        for s in range(T_OUT):
            nc.sync.dma_start(
                out=_dap(out, [[T_OUT, CHUNK]], off=b0 * T_OUT + s),
                in_=y_all[:, s, :],
            )


def build_nc(bc):
    from contextlib import ExitStack

    nc = bacc.Bacc("TRN2", target_bir_lowering=False, debug=False)
    enc = nc.dram_tensor("encoder_out", [T_IN, bc, H], F32, kind="ExternalInput").ap()
    hid = nc.dram_tensor("encoder_hid", [bc, H], F32, kind="ExternalInput").ap()
    last = nc.dram_tensor("last", [bc, OV], F32, kind="ExternalInput").ap()
    w_ih = nc.dram_tensor("W_ih", [3 * H, OV], F32, kind="ExternalInput").ap()
    w_hh = nc.dram_tensor("W_hh", [3 * H, H], F32, kind="ExternalInput").ap()
    b_ih = nc.dram_tensor("b_ih", [3 * H], F32, kind="ExternalInput").ap()
    b_hh = nc.dram_tensor("b_hh", [3 * H], F32, kind="ExternalInput").ap()
    w_lin = nc.dram_tensor("W_lin", [1, H], F32, kind="ExternalInput").ap()
    b_lin = nc.dram_tensor("b_lin", [1], F32, kind="ExternalInput").ap()
    out = nc.dram_tensor("out", [bc, T_OUT], F32, kind="ExternalOutput").ap()

    with tile.TileContext(nc) as tc:
        with ExitStack() as ctx:
            decoder_body(
                ctx, tc, enc, hid, last, w_ih, w_hh, b_ih, b_hh, w_lin, b_lin, out, bc
            )
    nc.compile()
    return nc


_CACHE = {}


def _shard_inputs(inputs, bc):
    wkeys = ["W_ih", "W_hh", "b_ih", "b_hh", "W_lin", "b_lin"]
    w = {k: np.ascontiguousarray(np.asarray(inputs[k], dtype=np.float32)) for k in wkeys}
    enc = np.asarray(inputs["encoder_out"], dtype=np.float32)
    hid = np.asarray(inputs["encoder_hid"], dtype=np.float32)
    last = np.asarray(inputs["last"], dtype=np.float32)
    in_maps = []
    ncores = enc.shape[1] // bc
    for c in range(ncores):
        sl = slice(c * bc, (c + 1) * bc)
        in_maps.append(
            {
                "encoder_out": np.ascontiguousarray(enc[:, sl, :]),
                "encoder_hid": np.ascontiguousarray(hid[sl]),
                "last": np.ascontiguousarray(last[sl]),
                **w,
            }
        )
    return in_maps


def kernel(**inputs):
    from concourse.bass_utils import run_bass_kernel_spmd

    bc = B // NCORES
    if bc not in _CACHE:
        _CACHE[bc] = build_nc(bc)
    nc = _CACHE[bc]
    in_maps = _shard_inputs(inputs, bc)
    res = run_bass_kernel_spmd(nc, in_maps, core_ids=list(range(NCORES)))
    return np.concatenate([r["out"] for r in res.results], axis=0)


# revision 16
# speedup vs baseline: 1.0691x; 1.0691x over previous
# Trainium2 Bass kernel for nn_Decoder (attention + GRUCell decode loop).
#
# Sharding: pure data parallel over the batch dim across 8 NeuronCores.
# Each core processes B/8 = 8192 batch elements; weights are replicated.
#
# Per-core layout strategy:
#   - batch processed in chunks of 512 (4 "waves" of 128 partitions)
#   - attention (scores/softmax/context) computed batch-major on DVE/ACT
#   - GRU matmuls on the PE in feature-major layout (weights stationary),
#     with PE transposes moving hidden between layouts
#   - the `last` feedback window is kept as a 3-row ring buffer so no
#     cross-partition shifts are ever needed (rotated W_ih stationaries)

import os
import numpy as np

import concourse.bacc as bacc
import concourse.bass as bass
import concourse.mybir as mybir
import concourse.tile as tile
from concourse.masks import make_identity

F32 = mybir.dt.float32
AX = mybir.AxisListType
OP = mybir.AluOpType
AF = mybir.ActivationFunctionType

T_IN = 12
T_OUT = 12
H = 64
OV = 3
B = 65536
NCORES = 8
P = 128  # sbuf partitions per wave
NW = 4  # waves per chunk
CHUNK = P * NW  # 512


def _dap(x: bass.AP, dims, off=0):
    """Custom access pattern over x's tensor: explicit [step, count] dims."""
    return bass.AP(tensor=x.tensor, offset=x.offset + off, ap=[list(d) for d in dims])


def _bap(t, dims):
    """AP over tile t keeping its partition dim, custom free dims."""
    a = t[:] if not isinstance(t, bass.AP) else t
    return bass.AP(
        tensor=a.tensor, offset=a.offset, ap=[list(a.ap[0])] + [list(d) for d in dims]
    )


def decoder_body(ctx, tc, enc, hid, last, w_ih, w_hh, b_ih, b_hh, w_lin, b_lin, out, bc):
    nc = tc.nc
    n_chunks = bc // CHUNK

    consts = ctx.enter_context(tc.tile_pool(name="consts", bufs=1))
    encp = ctx.enter_context(tc.tile_pool(name="encp", bufs=2))
    tmpp = ctx.enter_context(tc.tile_pool(name="tmpp", bufs=2))
    statep = ctx.enter_context(tc.tile_pool(name="statep", bufs=3))
    workp = ctx.enter_context(tc.tile_pool(name="workp", bufs=3))
    outp = ctx.enter_context(tc.tile_pool(name="outp", bufs=2))
    psump = ctx.enter_context(tc.tile_pool(name="psump", bufs=1, space="PSUM"))

    # ---------------- constants ----------------
    ident = consts.tile([P, P], F32, tag="ident")
    make_identity(nc, ident)

    # W_hh (192, 64) -> whhT (64, 192) so whhT[:, g*64:(g+1)*64] is the lhsT
    # (K=64, M=64) for gate g's hidden matmul.
    whhT = consts.tile([H, 3 * H], F32, tag="whhT")
    nc.sync.dma_start(out=whhT, in_=_dap(w_hh, [[1, H], [H, 3 * H]]))

    # W_ih (192, 3) -> three row-rotated copies of its transpose (3, 192).
    # At step s (m = s % 3) ring row r must be multiplied by W_ih column
    # (m - 1 - r) % 3.
    wihrot = consts.tile([OV, 3, 3 * H], F32, tag="wihrot")
    for m in range(3):
        for r in range(OV):
            i = (m - 1 - r) % 3
            nc.sync.dma_start(
                out=wihrot[r : r + 1, m, :],
                in_=_dap(w_ih, [[1, 1], [OV, 3 * H]], off=i),
            )

    # biases as (64, 3) feature-major tiles: column g = gate g
    bihs = consts.tile([H, 3], F32, tag="bihs")
    nc.sync.dma_start(out=bihs, in_=_dap(b_ih, [[1, H], [H, 3]]))
    bhhs = consts.tile([H, 3], F32, tag="bhhs")
    nc.sync.dma_start(out=bhhs, in_=_dap(b_hh, [[1, H], [H, 3]]))


    # W_lin (1, 64) -> (64, 1), plus 3 rotated (64, 3) variants with the
    # column placed at ring slot m (others zero) for the ring update matmul.
    wlinT = consts.tile([H, 1], F32, tag="wlinT")
    nc.sync.dma_start(out=wlinT, in_=_dap(w_lin, [[1, H], [H, 1]]))
    wlin3 = consts.tile([H, 3, 3], F32, tag="wlin3")
    nc.vector.memset(wlin3, 0.0)
    for m in range(3):
        nc.vector.tensor_copy(out=wlin3[:, m, m : m + 1], in_=wlinT)

    blin = consts.tile([1, 1], F32, tag="blin")
    nc.sync.dma_start(out=blin, in_=_dap(b_lin, [[1, 1], [1, 1]]))
    # b_lin replicated on partitions 1..2 (bias APs must match the in_
    # partition range when reading ring-psum rows 1/2)
    blin3g = consts.tile([2, 1], F32, tag="blin3g")
    nc.sync.dma_start(out=blin3g, in_=_dap(b_lin, [[0, 2], [1, 1]]))

    # The ring stores y WITHOUT b_lin (so the ring update is a single fused
    # op).  The missing b_lin is folded into the gate biases: at step s the
    # ring holds min(s, 3) y-type entries, each contributing
    # b_lin * W_ih[:, age] to the pre-activations.
    # bias_var[k, v, g]: v = min(s, 3); g in {r, z, n}; for r/z it includes
    # b_ih+b_hh, for n only b_ih (b_hh_n is applied in the hn Identity op).
    blin64 = consts.tile([H, 1], F32, tag="blin64")
    nc.sync.dma_start(out=blin64, in_=_dap(b_lin, [[0, H], [1, 1]]))
    wcol = consts.tile([H, OV, 3], F32, tag="wcol")  # [k, age i, gate]
    for i in range(OV):
        nc.sync.dma_start(
            out=wcol[:, i, :], in_=_dap(w_ih, [[OV, H], [OV * H, 3]], off=i)
        )
    bias_var = consts.tile([H, 4, 3], F32, tag="bias_var")
    nc.vector.tensor_copy(out=bias_var[:, 0, :], in_=bihs)
    nc.vector.tensor_add(
        out=bias_var[:, 0, 0:2], in0=bias_var[:, 0, 0:2], in1=bhhs[:, 0:2]
    )
    for v in range(1, 4):
        nc.vector.scalar_tensor_tensor(
            out=bias_var[:, v, :],
            in0=wcol[:, v - 1, :],
            scalar=blin64,
            in1=bias_var[:, v - 1, :],
            op0=OP.mult,
            op1=OP.add,
        )
    # r/z biases pre-halved: sigmoid(x+b) = 0.5 + 0.5*tanh(0.5*x + 0.5*b),
    # keeping ACT on the exp/tanh table set (no per-step table reloads).
    for v in range(4):
        nc.vector.tensor_scalar_mul(
            out=bias_var[:, v, 0:2], in0=bias_var[:, v, 0:2], scalar1=0.5
        )

    # mask3[:, m]: 0 at slot m, 1 elsewhere (ring update kills old slot m)
    mask3 = consts.tile([OV, 3], F32, tag="mask3")
    nc.gpsimd.memset(mask3, 1.0)
    # mask3[x, y] = (x - y) != 0 ? 1.0 : 0.0
    nc.gpsimd.affine_select(
        out=mask3,
        in_=mask3,
        compare_op=OP.not_equal,
        fill=0.0,
        base=0,
        pattern=[[-1, 3]],
        channel_multiplier=1,
    )

    # ---------------- main loop ----------------
    reps = int(os.environ.get("BENCH_REPS", "1"))
    for _rep in range(reps):
      for c in range(n_chunks):
        b0 = c * CHUNK

        # enc chunk: (128 p, 4 w, 12 t, 64 k); batch index = b0 + w*128 + p
        enc_b = encp.tile([P, NW, T_IN, H], F32, tag="enc")
        for w in range(NW):
            nc.sync.dma_start(
                out=enc_b[:, w, :, :],
                in_=_dap(
                    enc, [[H, P], [bc * H, T_IN], [1, H]], off=(b0 + w * P) * H
                ),
            )

        # initial hidden, batch-major (128, 4, 64)
        h_b = statep.tile([P, NW, H], F32, tag="h_b")
        nc.sync.dma_start(
            out=h_b, in_=_dap(hid, [[H, P], [P * H, NW], [1, H]], off=b0 * H)
        )
        # initial hidden, feature-major (64, 512)
        ps_t0 = psump.tile([H, CHUNK], F32, tag="ps_hT")
        for w in range(NW):
            nc.tensor.transpose(ps_t0[:, w * P : (w + 1) * P], h_b[:, w, :], ident)
        hT = statep.tile([H, CHUNK], F32, tag="hT")
        nc.scalar.copy(out=hT, in_=ps_t0)

        # last-window ring (3, 512): ring[(-1 - i) % 3] = last[:, i]
        lst = statep.tile([OV, CHUNK], F32, tag="lst")
        for i in range(OV):
            slot = (-1 - i) % 3
            nc.sync.dma_start(
                out=lst[slot : slot + 1, :],
                in_=_dap(last, [[1, 1], [OV, CHUNK]], off=b0 * OV + i),
            )

        y_all = outp.tile([1, T_OUT, CHUNK], F32, tag="y_all")

        for s in range(T_OUT):
            m = s % 3
            bv = min(s, 3)

            # ---- attention scores: s[p,w,t] = sum_k enc*h ----
            # (multiplies on GpSimd, reductions on DVE — DVE is the
            # bottleneck engine, POOL is otherwise idle)
            tmp = tmpp.tile([P, NW, T_IN, H], F32, tag="tmp")
            nc.vector.tensor_mul(
                out=tmp[:, 0:2],
                in0=enc_b[:, 0:2],
                in1=_bap(h_b[:, 0:2, :], [[H, 2], [0, T_IN], [1, H]]),
            )
            _meng = nc.gpsimd if os.environ.get("ATTN_POOL", "1") == "1" else nc.vector
            _meng.tensor_mul(
                out=tmp[:, 2:4],
                in0=enc_b[:, 2:4],
                in1=_bap(h_b[:, 2:4, :], [[H, 2], [0, T_IN], [1, H]]),
            )
            sc = workp.tile([P, NW, T_IN], F32, tag="sc")
            nc.vector.tensor_reduce(out=sc, in_=tmp, axis=AX.X, op=OP.add)

            # ---- softmax over t ----
            nmax = workp.tile([P, NW], F32, tag="nmax")
            nc.vector.tensor_reduce(
                out=nmax, in_=sc, axis=AX.X, op=OP.max, negate=True
            )
            e = workp.tile([P, NW, T_IN], F32, tag="e")
            nc.vector.tensor_add(
                out=e, in0=sc, in1=_bap(nmax, [[1, NW], [0, T_IN]])
            )
            nc.scalar.activation(out=e, in_=e, func=AF.Exp)
            ssum = workp.tile([P, NW], F32, tag="ssum")
            nc.vector.tensor_reduce(out=ssum, in_=e, axis=AX.X, op=OP.add)
            nc.vector.reciprocal(out=ssum, in_=ssum)
            wgt = workp.tile([P, NW, T_IN], F32, tag="wgt")
            nc.vector.tensor_mul(
                out=wgt, in0=e, in1=_bap(ssum, [[1, NW], [0, T_IN]])
            )

            # ---- context: ctx[p,w,k] = sum_t enc * wgt ----
            # ---- context, fused with the transpose to feature-major ----
            # q[p,w,t,k] = wgt * enc (elementwise); the sum over t is done by
            # the PE: 12 transpose-matmuls per wave accumulate q_t^T into the
            # same PSUM columns, yielding ctx^T (64, 512) with no DVE reduce.
            tmp2 = tmpp.tile([P, NW, T_IN, H], F32, tag="tmp")
            _meng.tensor_mul(
                out=tmp2,
                in0=enc_b,
                in1=_bap(wgt, [[T_IN, NW], [1, T_IN], [0, H]]),
            )
            ps_ct = psump.tile([H, CHUNK], F32, tag="ps_hT")
            for w in range(NW):
                for t in range(T_IN):
                    nc.tensor.matmul(
                        ps_ct[:, w * P : (w + 1) * P],
                        tmp2[:, w, t, :],
                        ident,
                        start=(t == 0),
                        stop=(t == T_IN - 1),
                        is_transpose=True,
                    )
            hattT = statep.tile([H, CHUNK], F32, tag="hattT")
            nc.vector.tensor_add(out=hattT, in0=hT, in1=ps_ct)

            # ---- GRU gates ----
            ps_r = psump.tile([H, CHUNK], F32, tag="ps_r")
            nc.tensor.matmul(ps_r, whhT[:, 0:H], hattT, start=True, stop=False)
            nc.tensor.matmul(ps_r, wihrot[:, m, 0:H], lst, start=False, stop=True)
            ps_z = psump.tile([H, CHUNK], F32, tag="ps_z")
            nc.tensor.matmul(ps_z, whhT[:, H : 2 * H], hattT, start=True, stop=False)
            nc.tensor.matmul(
                ps_z, wihrot[:, m, H : 2 * H], lst, start=False, stop=True
            )
            r_s = workp.tile([H, CHUNK], F32, tag="r_s")
            nc.scalar.activation(
                out=r_s, in_=ps_r, func=AF.Tanh, bias=bias_var[:, bv, 0:1], scale=0.5
            )
            nc.gpsimd.tensor_scalar(
                out=r_s, in0=r_s, scalar1=0.5, scalar2=0.5, op0=OP.mult, op1=OP.add
            )
            z_s = workp.tile([H, CHUNK], F32, tag="z_s")
            nc.scalar.activation(
                out=z_s, in_=ps_z, func=AF.Tanh, bias=bias_var[:, bv, 1:2], scale=0.5
            )
            nc.gpsimd.tensor_scalar(
                out=z_s, in0=z_s, scalar1=0.5, scalar2=0.5, op0=OP.mult, op1=OP.add
            )

            ps_n2 = psump.tile([H, CHUNK], F32, tag="ps_n2")
            nc.tensor.matmul(
                ps_n2, whhT[:, 2 * H : 3 * H], hattT, start=True, stop=True
            )
            ps_n1 = psump.tile([H, CHUNK], F32, tag="ps_n1")
            nc.tensor.matmul(
                ps_n1, wihrot[:, m, 2 * H : 3 * H], lst, start=True, stop=True
            )
            hn = workp.tile([H, CHUNK], F32, tag="hn")
            nc.scalar.activation(
                out=hn, in_=ps_n2, func=AF.Identity, bias=bhhs[:, 2:3], scale=1.0
            )
            u = workp.tile([H, CHUNK], F32, tag="u")
            nc.vector.tensor_mul(out=u, in0=r_s, in1=hn)
            nc.vector.tensor_add(out=u, in0=u, in1=ps_n1)
            n_t = workp.tile([H, CHUNK], F32, tag="n_t")
            nc.scalar.activation(
                out=n_t, in_=u, func=AF.Tanh, bias=bias_var[:, bv, 2:3], scale=1.0
            )
            # h' = n + z * (h_att - n)
            v = workp.tile([H, CHUNK], F32, tag="v")
            nc.vector.tensor_sub(out=v, in0=hattT, in1=n_t)
            nc.vector.tensor_mul(out=v, in0=z_s, in1=v)
            hT_new = statep.tile([H, CHUNK], F32, tag="hT")
            nc.vector.tensor_add(out=hT_new, in0=n_t, in1=v)

            # ---- output y = h' @ W_lin.T + b_lin ----
            ps_y = psump.tile([1, CHUNK], F32, tag="ps_y")
            nc.tensor.matmul(ps_y, wlinT, hT_new, start=True, stop=True)
            nc.scalar.activation(
                out=y_all[:, s, :], in_=ps_y, func=AF.Identity, bias=blin, scale=1.0
            )

            # ---- ring update: slot m <- y ----
            ps_g = psump.tile([OV, CHUNK], F32, tag="ps_g")
            nc.tensor.matmul(ps_g, wlin3[:, m, :], hT_new, start=True, stop=True)
            lst_new = statep.tile([OV, CHUNK], F32, tag="lst")
            nc.vector.scalar_tensor_tensor(
                out=lst_new,
                in0=lst,
                scalar=mask3[:, m : m + 1],
                in1=ps_g,
                op0=OP.mult,
                op1=OP.add,
            )

            # ---- h' back to batch-major for next step's scores ----
            if s < T_OUT - 1:
                ps_hb = psump.tile([P, NW * H], F32, tag="ps_hb")
                for w in range(NW):
                    nc.tensor.transpose(
                        ps_hb[:, w * H : (w + 1) * H],
                        hT_new[:, w * P : (w + 1) * P],
                        ident[0:H, 0:H],
                    )
                h_b = statep.tile([P, NW, H], F32, tag="h_b")
                nc.scalar.copy(out=h_b, in_=ps_hb)

            hT = hT_new
            lst = lst_new

        # ---- write chunk outputs: y_all (1, 12, 512) -> out[b0:b0+512, :] ----
        for s in range(T_OUT):
            nc.sync.dma_start(
                out=_dap(out, [[T_OUT, CHUNK]], off=b0 * T_OUT + s),
                in_=y_all[:, s, :],
            )


def build_nc(bc):
    from contextlib import ExitStack

    nc = bacc.Bacc("TRN2", target_bir_lowering=False, debug=False)
    enc = nc.dram_tensor("encoder_out", [T_IN, bc, H], F32, kind="ExternalInput").ap()
    hid = nc.dram_tensor("encoder_hid", [bc, H], F32, kind="ExternalInput").ap()
    last = nc.dram_tensor("last", [bc, OV], F32, kind="ExternalInput").ap()
    w_ih = nc.dram_tensor("W_ih", [3 * H, OV], F32, kind="ExternalInput").ap()
    w_hh = nc.dram_tensor("W_hh", [3 * H, H], F32, kind="ExternalInput").ap()
    b_ih = nc.dram_tensor("b_ih", [3 * H], F32, kind="ExternalInput").ap()
    b_hh = nc.dram_tensor("b_hh", [3 * H], F32, kind="ExternalInput").ap()
    w_lin = nc.dram_tensor("W_lin", [1, H], F32, kind="ExternalInput").ap()
    b_lin = nc.dram_tensor("b_lin", [1], F32, kind="ExternalInput").ap()
    out = nc.dram_tensor("out", [bc, T_OUT], F32, kind="ExternalOutput").ap()

    with tile.TileContext(nc) as tc:
        with ExitStack() as ctx:
            decoder_body(
                ctx, tc, enc, hid, last, w_ih, w_hh, b_ih, b_hh, w_lin, b_lin, out, bc
            )
    nc.compile()
    return nc


_CACHE = {}


def _shard_inputs(inputs, bc):
    wkeys = ["W_ih", "W_hh", "b_ih", "b_hh", "W_lin", "b_lin"]
    w = {k: np.ascontiguousarray(np.asarray(inputs[k], dtype=np.float32)) for k in wkeys}
    enc = np.asarray(inputs["encoder_out"], dtype=np.float32)
    hid = np.asarray(inputs["encoder_hid"], dtype=np.float32)
    last = np.asarray(inputs["last"], dtype=np.float32)
    in_maps = []
    ncores = enc.shape[1] // bc
    for c in range(ncores):
        sl = slice(c * bc, (c + 1) * bc)
        in_maps.append(
            {
                "encoder_out": np.ascontiguousarray(enc[:, sl, :]),
                "encoder_hid": np.ascontiguousarray(hid[sl]),
                "last": np.ascontiguousarray(last[sl]),
                **w,
            }
        )
    return in_maps


def kernel(**inputs):
    from concourse.bass_utils import run_bass_kernel_spmd

    bc = B // NCORES
    if bc not in _CACHE:
        _CACHE[bc] = build_nc(bc)
    nc = _CACHE[bc]
    in_maps = _shard_inputs(inputs, bc)
    res = run_bass_kernel_spmd(nc, in_maps, core_ids=list(range(NCORES)))
    return np.concatenate([r["out"] for r in res.results], axis=0)
